# revision 22
# baseline (speedup 1.0000x reference)
"""Trainium2 Bass kernel for causal self-attention (B=2, S=2048, D=1024, H=16).

Sharding: 8 cores = 2 batches x 4 head-groups. Core c handles batch c//4 and
heads 4*(c%4) .. 4*(c%4)+4. Each core receives its batch's x [2048, 1024] and
its [1024, 768] slice of w_qkv (q/k/v columns for its 4 heads), and produces
the [2048, 256] output slice. No cross-core communication is needed; the host
gathers the slices. w_o is unused by the reference (no output projection).

Per-core kernel (Tile framework), fp16 matmul path with fp32 psum/softmax.
v2: software-pipelined emission. The scalar engine's exp (softmax) and the PE
matmuls are the two dominant costs (~85us and ~95us busy); v1 serialized them
(score -> exp -> AV per j-block with the PE stalled during exp). v2 keeps the
PE busy during exp by (a) issuing the next j-block's score matmul before the
previous block's AV, and (b) draining a background queue of projection matmul
chains (for later query chunks) between attention matmuls. Exp work on the
diagonal j-blocks is trimmed to the unmasked column range via strided APs.
"""

import sys

sys.path.insert(0, "/opt/trn_rl_repo")

from collections import deque
from contextlib import ExitStack

import numpy as np

import concourse.bass as bass
import concourse.tile as tile
from concourse import bacc, masks, mybir
from concourse.bass_utils import run_bass_kernel_spmd

B, S, D, H = 2, 2048, 1024, 16
HD = 64          # head dim
HPC = 4          # heads per core
NCORES = 8
P = 128
NS = S // P      # 16 s-blocks
KC = D // P      # 8 d-chunks
CH = 512         # query-chunk width
NT = S // CH     # 4 query chunks
COLS = 3 * HPC * HD   # 768 projection columns per core
F32 = mybir.dt.float32
F16 = mybir.dt.float16
SCALE = 1.0 / np.sqrt(HD)

PSUM = bass.MemorySpace.PSUM


def _build_body(ctx: ExitStack, tc: "tile.TileContext", x_d, w_d, o_d):
    nc = tc.nc

    persist = ctx.enter_context(tc.tile_pool(name="persist", bufs=1))
    ident_h = persist.tile([P, P], F16)
    masks.make_identity(nc, ident_h[:])

    # v in natural layout + ones column, per (j-block, head): [128, 65] slices
    vaug = persist.tile([P, NS * HPC * 65], F16)
    nc.vector.memset(
        vaug[:].rearrange("p (n c) -> p n c", c=65)[:, :, 64:65], 1.0
    )
    # final output staging [128, 16 i-blocks * 4 heads * 64]
    out_sb = persist.tile([P, NS * HPC * HD], F32)
    w_all = persist.tile([P, KC * COLS], F16)     # [128, 8*768]
    xT = persist.tile([P, KC * S], F16)           # [128, 8*2048]
    qkT = persist.tile([P, 4 * S], F16)   # m0,m1 = q(h01,h23); m2,m3 = k

    # Force the exp table load to happen during kernel init, before the
    # scalar queue picks up its share of the input DMAs.
    actwarm = persist.tile([1, 1], F32)
    nc.scalar.activation(
        actwarm[:], ident_h[0:1, 0:1], mybir.ActivationFunctionType.Exp
    )


    # ---- all input DMAs up front ----------------------------------------
    # Ramp-critical data (w + x chunk 0) is split across both HWDGE queues
    # (sync gets w, scalar gets x-g0) so the first projection chain's k-th
    # matmul unblocks after k+1 DMAs on each queue. The remaining x chunks
    # arrive as one merged [128, 1536] DMA per k-block on the sync queue.
    for k in range(KC):
        nc.sync.dma_start(
            w_all[:, k * COLS:(k + 1) * COLS],
            w_d[k * P:(k + 1) * P, :],
        )
        nc.scalar.dma_start(
            xT[:, k * S: k * S + CH],
            x_d[k * P:(k + 1) * P, 0:CH],
        )
    for k in range(KC):
        nc.sync.dma_start(
            xT[:, k * S + CH: (k + 1) * S],
            x_d[k * P:(k + 1) * P, CH:S],
        )

    ps_st = ctx.enter_context(tc.tile_pool(name="ps_st", bufs=2, space=PSUM))
    ps_o = ctx.enter_context(tc.tile_pool(name="ps_o", bufs=2, space=PSUM))
    ps_small = ctx.enter_context(tc.tile_pool(name="ps_small", bufs=2, space=PSUM))
    pp = ctx.enter_context(tc.tile_pool(name="p", bufs=6))
    osbp = ctx.enter_context(tc.tile_pool(name="osb", bufs=3))
    finp = ctx.enter_context(tc.tile_pool(name="fin", bufs=3))
    rcp = ctx.enter_context(tc.tile_pool(name="rcol", bufs=4))

    # HAM warmer: ~5us of back-to-back dummy matmuls on the identity while
    # the input DMAs run, so the PE clock gate is already at 8/8 (2.4 GHz)
    # when the first projection chain issues. Costs nothing: the PE would
    # otherwise idle during the DMA ramp.
    for _ in range(3):
        ham_ps = ps_small.tile([P, P], F32, tag="small", name="ham_ps")
        for _ in range(16):
            nc.tensor.matmul(
                ham_ps[:], ident_h[:], ident_h[:], start=True, stop=True
            )
    # zero the three osb slots once, so the transpose DMAs never read
    # uninitialized SBUF in the pad rows (65..79)
    for _ in range(3):
        osb_init = osbp.tile([80, CH], F16, tag="osb", name="osb_init")
        nc.vector.memset(osb_init[:], 0.0)

    # ---- background work: projection chains, emitted in small units ------
    # Each chain is split into units of 4 matmuls; the last unit also emits
    # the PSUM->SBUF cast. Units are drained between attention matmuls so
    # the PE has independent work while the scalar engine runs exp.
    bg = deque()          # of (emit_fn, chain_name, is_last_unit)
    done = set()

    def add_chain(name, units):
        for i, u in enumerate(units):
            bg.append((u, name, i == len(units) - 1))

    def drain(n):
        for _ in range(n):
            if not bg:
                return
            u, nm, last = bg.popleft()
            u()
            if last:
                done.add(nm)

    def require(name):
        while name not in done:
            assert bg, f"chain {name} needed but not queued"
            u, nm, last = bg.popleft()
            u()
            if last:
                done.add(nm)

    def proj_chain(m, t):
        """qkT[:, m*S + t-chunk] = (w col-block m)^T @ x^T, as 2 units."""
        st = {}

        def u1():
            st["ps"] = ps_small.tile([P, CH], F32, tag="small", name="pp_ps")
            for k in range(4):
                nc.tensor.matmul(
                    st["ps"][:],
                    w_all[:, k * COLS + m * P: k * COLS + (m + 1) * P],
                    xT[:, k * S + t * CH: k * S + (t + 1) * CH],
                    start=(k == 0),
                    stop=False,
                )

        def u2():
            for k in range(4, KC):
                nc.tensor.matmul(
                    st["ps"][:],
                    w_all[:, k * COLS + m * P: k * COLS + (m + 1) * P],
                    xT[:, k * S + t * CH: k * S + (t + 1) * CH],
                    start=False,
                    stop=(k == KC - 1),
                )
            nc.vector.tensor_copy(
                qkT[:, m * S + t * CH: m * S + (t + 1) * CH], st["ps"][:]
            )

        return [u1, u2]

    def v_chain(sb):
        """vaug[s-block sb] = x[sb] @ w_v (natural layout), as 2 units."""
        st = {}

        def u1():
            st["ps"] = ps_small.tile([P, CH], F32, tag="small", name="pv")
            for k in range(4):
                nc.tensor.matmul(
                    st["ps"][:, 0:256],
                    xT[:, k * S + sb * P: k * S + (sb + 1) * P],
                    w_all[:, k * COLS + 512: k * COLS + 768],
                    start=(k == 0),
                    stop=False,
                )

        def u2():
            for k in range(4, KC):
                nc.tensor.matmul(
                    st["ps"][:, 0:256],
                    xT[:, k * S + sb * P: k * S + (sb + 1) * P],
                    w_all[:, k * COLS + 512: k * COLS + 768],
                    start=False,
                    stop=(k == KC - 1),
                )
            nc.vector.tensor_copy(
                vaug[:, sb * HPC * 65:(sb + 1) * HPC * 65]
                .rearrange("p (g c) -> p g c", c=65)[:, :, 0:64],
                st["ps"][:, 0:256].rearrange("p (g c) -> p g c", c=64),
            )

        return [u1, u2]

    out_view = out_sb[:].rearrange("p (i g d) -> p i g d", g=HPC, d=HD)

    def finalize_head(h, t, po_h):
        """Transpose outT to natural layout (via the DMA XBAR, keeping the
        PE out of it), divide by the denominator."""
        osb_t = osbp.tile([80, CH], F16, tag="osb")
        nc.vector.tensor_copy(osb_t[0:65, :], po_h[:])
        fin_sb = finp.tile([P, 4 * 80], F16, tag="fin")
        for b in range(4):
            nc.sync.dma_start_transpose(
                fin_sb[:, b * 80:(b + 1) * 80], osb_t[:, b * P:(b + 1) * P]
            )
        fin_view = fin_sb[:].rearrange("p (n c) -> p n c", c=80)
        rc = rcp.tile([P, 4], F32, tag="rc")
        nc.vector.reciprocal(rc[:], fin_view[:, :, 64])
        nc.vector.tensor_mul(
            out_view[:, 4 * t:4 * t + 4, h, :],
            fin_view[:, :, 0:64],
            rc[:].broadcast_to([P, 4, HD]),
        )

    def attn_pair(pair, t, budget, carry):
        """Heads 2*pair, 2*pair+1; query chunk t (i in [512t, 512t+512)).

        `carry` is a list of closures (previous pair's finalize + output
        DMAs) run after this pair's first score/exp is emitted, so the
        scalar engine starts the next exp stream during the finalize."""
        hA, hB = 2 * pair, 2 * pair + 1
        qm, km = pair, 2 + pair
        require(f"q{pair}t{t}")
        po_a = ps_o.tile([65, CH], F32, tag="o")
        po_b = ps_o.tile([65, CH], F32, tag="o")
        po = {hA: po_a, hB: po_b}
        njb = 4 * t + 4
        pending = deque()  # (jb, p_t) AVs, emitted two slots behind

        def emit_av(jb, p_t, last):
            doff = jb - 4 * t
            off = max(0, P * doff)
            require(f"v{jb}")
            for hi, h in enumerate((hA, hB)):
                nc.tensor.matmul(
                    po[h][:, off:CH],
                    vaug[:, (jb * HPC + h) * 65: (jb * HPC + h + 1) * 65],
                    p_t[:, hi * CH + off:(hi + 1) * CH],
                    start=(jb == 0),
                    stop=last,
                )

        for jb in range(njb):
            if jb >= 4 * t:
                require(f"k{pair}t{t}")
            st = ps_st.tile([P, 2 * CH], F32, tag="st")
            for hi, h in enumerate((hA, hB)):
                hb = (h % 2) * 64
                nc.tensor.matmul(
                    st[:, hi * CH:(hi + 1) * CH],
                    qkT[hb:hb + 64, km * S + jb * P: km * S + (jb + 1) * P],
                    qkT[hb:hb + 64, qm * S + t * CH: qm * S + (t + 1) * CH],
                    start=True,
                    stop=True,
                    tile_position=(hb, 0),
                )
            p_t = pp.tile([P, 2 * CH], F16, tag="p")
            doff = jb - 4 * t
            off = max(0, P * doff)
            if off > 0:
                # skip exp of the fully-masked leading columns of both heads
                st_v = st[:].rearrange("p (h w) -> p h w", h=2)[:, :, off:CH]
                p_v = p_t[:].rearrange("p (h w) -> p h w", h=2)[:, :, off:CH]
            else:
                st_v, p_v = st[:], p_t[:]
            nc.scalar.activation(
                p_v, st_v, mybir.ActivationFunctionType.Exp,
                scale=float(SCALE),
            )
            if doff >= 0:
                # triangular mask on the diagonal 128x128 sub-block only;
                # the fully-masked zone is skipped by exp + AV column ranges.
                for hi in range(2):
                    c0 = hi * CH + off
                    sl = p_t[:, c0:c0 + P]
                    nc.gpsimd.affine_select(
                        out=sl,
                        in_=sl,
                        compare_op=mybir.AluOpType.is_ge,
                        fill=0.0,
                        base=0,
                        channel_multiplier=-1,
                        pattern=[[1, P]],
                    )
            if jb == 0:
                for f in carry:
                    f()
                carry = []
            drain(budget)
            pending.append((jb, p_t))
            if len(pending) > 2:
                ajb, ap_t = pending.popleft()
                emit_av(ajb, ap_t, last=False)
        while pending:
            ajb, ap_t = pending.popleft()
            emit_av(ajb, ap_t, last=(not pending))
        return [
            (lambda h=h: finalize_head(h, t, po[h])) for h in (hA, hB)
        ]

    # ---- prologue: chains needed to start attn(pair0, t0) ----------------
    add_chain("q0t0", proj_chain(0, 0))
    add_chain("k0t0", proj_chain(2, 0))
    add_chain("v0", v_chain(0))
    require("q0t0")
    require("k0t0")
    require("v0")

    carry = []
    for t in range(NT):
        # this phase's own late-deadline chains (diag v blocks, pair-1 q/k)
        for sb in range(4 * t + (1 if t == 0 else 0), 4 * t + 4):
            add_chain(f"v{sb}", v_chain(sb))
        add_chain(f"q1t{t}", proj_chain(1, t))
        add_chain(f"k1t{t}", proj_chain(3, t))
        # next phase's pair-0 q/k (needed at its very first slot)
        if t + 1 < NT:
            add_chain(f"q0t{t + 1}", proj_chain(0, t + 1))
            add_chain(f"k0t{t + 1}", proj_chain(2, t + 1))
        budget = 1
        carry = attn_pair(0, t, budget, carry)
        carry = attn_pair(1, t, budget, carry)

        def out_dma(t=t):
            for b in range(4):
                ib = 4 * t + b
                nc.sync.dma_start(
                    o_d[ib * P:(ib + 1) * P, :],
                    out_sb[:, ib * HPC * HD:(ib + 1) * HPC * HD],
                )

        carry.append(out_dma)
    for f in carry:
        f()
    assert not bg, f"{len(bg)} background units left unemitted"


def build_program():
    nc = bacc.Bacc(
        "TRN2",
        target_bir_lowering=False,
        debug=False,
        enable_asserts=False,
    )
    x_d = nc.dram_tensor("x", [D, S], F16, kind="ExternalInput").ap()
    w_d = nc.dram_tensor("w", [D, COLS], F16, kind="ExternalInput").ap()
    o_d = nc.dram_tensor("o", [S, HPC * HD], F32, kind="ExternalOutput").ap()

    with tile.TileContext(nc) as tc, ExitStack() as ctx:
        _build_body(ctx, tc, x_d, w_d, o_d)
    nc.compile()
    return nc


_CACHE = {}


def _compiled():
    if "nc" not in _CACHE:
        _CACHE["nc"] = build_program()
    return _CACHE["nc"]


def make_in_maps(x, w_qkv):
    x = np.asarray(x, dtype=np.float32)
    w_qkv = np.asarray(w_qkv, dtype=np.float32)
    # one transpose+cast per batch; cores sharing a batch reuse the array
    xT16 = [x[b].T.astype(np.float16) for b in range(B)]
    in_maps = []
    for c in range(NCORES):
        b = c // 4
        cs = (c % 4) * HPC * HD
        w_slice = np.concatenate(
            [
                w_qkv[:, cs:cs + HPC * HD],
                w_qkv[:, D + cs:D + cs + HPC * HD],
                w_qkv[:, 2 * D + cs:2 * D + cs + HPC * HD],
            ],
            axis=1,
        )
        in_maps.append(
            {
                "x": xT16[b],
                "w": np.ascontiguousarray(w_slice).astype(np.float16),
            }
        )
    return in_maps


def gather_out(results):
    out = np.empty((B, S, D), np.float32)
    for c in range(NCORES):
        b = c // 4
        cs = (c % 4) * HPC * HD
        out[b][:, cs:cs + HPC * HD] = results[c]["o"]
    return out


def kernel(x, w_qkv, w_o=None, **_):
    nc = _compiled()
    res = run_bass_kernel_spmd(nc, make_in_maps(x, w_qkv), core_ids=list(range(NCORES)))
    return gather_out(res.results)


# revision 25
# speedup vs baseline: 1.2721x; 1.2721x over previous
"""Trainium2 Bass kernel for causal self-attention (B=2, S=2048, D=1024, H=16).

Sharding: 8 cores = 2 batches x 4 head-groups. Core c handles batch c//4 and
heads 4*(c%4) .. 4*(c%4)+4. Each core receives its batch's x [2048, 1024] and
its [1024, 768] slice of w_qkv (q/k/v columns for its 4 heads), and produces
the [2048, 256] output slice. No cross-core communication is needed; the host
gathers the slices. w_o is unused by the reference (no output projection).

Per-core kernel (Tile framework), fp16 matmul path with fp32 psum/softmax.
v2: software-pipelined emission. The scalar engine's exp (softmax) and the PE
matmuls are the two dominant costs (~85us and ~95us busy); v1 serialized them
(score -> exp -> AV per j-block with the PE stalled during exp). v2 keeps the
PE busy during exp by (a) issuing the next j-block's score matmul before the
previous block's AV, and (b) draining a background queue of projection matmul
chains (for later query chunks) between attention matmuls. Exp work on the
diagonal j-blocks is trimmed to the unmasked column range via strided APs.
"""

import sys

sys.path.insert(0, "/opt/trn_rl_repo")

from collections import deque
from contextlib import ExitStack

import numpy as np

import concourse.bass as bass
import concourse.tile as tile
from concourse import bacc, masks, mybir
from concourse.bass_utils import run_bass_kernel_spmd

B, S, D, H = 2, 2048, 1024, 16
HD = 64          # head dim
HPC = 4          # heads per core
NCORES = 8
P = 128
NS = S // P      # 16 s-blocks
KC = D // P      # 8 d-chunks
CH = 512         # query-chunk width
NT = S // CH     # 4 query chunks
COLS = 3 * HPC * HD   # 768 projection columns per core
F32 = mybir.dt.float32
F16 = mybir.dt.float16
SCALE = 1.0 / np.sqrt(HD)

PSUM = bass.MemorySpace.PSUM


def _build_body(ctx: ExitStack, tc: "tile.TileContext", x_d, w_d, o_d):
    nc = tc.nc

    persist = ctx.enter_context(tc.tile_pool(name="persist", bufs=1))
    ident_h = persist.tile([P, P], F16)
    masks.make_identity(nc, ident_h[:])

    # v in natural layout + ones column, per (j-block, head): [128, 65] slices
    vaug = persist.tile([P, NS * HPC * 65], F16)
    nc.vector.memset(
        vaug[:].rearrange("p (n c) -> p n c", c=65)[:, :, 64:65], 1.0
    )
    # final output staging [128, 16 i-blocks * 4 heads * 64]
    out_sb = persist.tile([P, NS * HPC * HD], F32)
    w_all = persist.tile([P, KC * COLS], F16)     # [128, 8*768]
    xT = persist.tile([P, KC * S], F16)           # [128, 8*2048]
    qkT = persist.tile([P, 4 * S], F16)   # m0,m1 = q(h01,h23); m2,m3 = k

    # Force the exp table load to happen during kernel init, before the
    # scalar queue picks up its share of the input DMAs.
    actwarm = persist.tile([1, 1], F32)
    nc.scalar.activation(
        actwarm[:], ident_h[0:1, 0:1], mybir.ActivationFunctionType.Exp
    )


    # ---- all input DMAs up front ----------------------------------------
    # Ramp-critical data (w + x chunk 0) is split across both HWDGE queues
    # (sync gets w, scalar gets x-g0) so the first projection chain's k-th
    # matmul unblocks after k+1 DMAs on each queue. The remaining x chunks
    # arrive as one merged [128, 1536] DMA per k-block on the sync queue.
    for k in range(KC):
        nc.sync.dma_start(
            w_all[:, k * COLS:(k + 1) * COLS],
            w_d[k * P:(k + 1) * P, :],
        )
        nc.scalar.dma_start(
            xT[:, k * S: k * S + CH],
            x_d[k * P:(k + 1) * P, 0:CH],
        )
    for k in range(KC):
        nc.sync.dma_start(
            xT[:, k * S + CH: (k + 1) * S],
            x_d[k * P:(k + 1) * P, CH:S],
        )

    ps_st = ctx.enter_context(tc.tile_pool(name="ps_st", bufs=2, space=PSUM))
    ps_o = ctx.enter_context(tc.tile_pool(name="ps_o", bufs=2, space=PSUM))
    ps_small = ctx.enter_context(tc.tile_pool(name="ps_small", bufs=2, space=PSUM))
    pp = ctx.enter_context(tc.tile_pool(name="p", bufs=6))
    osbp = ctx.enter_context(tc.tile_pool(name="osb", bufs=3))
    rcp = ctx.enter_context(tc.tile_pool(name="rcol", bufs=4))

    # HAM warmer: ~5us of back-to-back dummy matmuls on the identity while
    # the input DMAs run, so the PE clock gate is already at 8/8 (2.4 GHz)
    # when the first projection chain issues. Costs nothing: the PE would
    # otherwise idle during the DMA ramp.
    for _ in range(3):
        ham_ps = ps_small.tile([P, P], F32, tag="small", name="ham_ps")
        for _ in range(16):
            nc.tensor.matmul(
                ham_ps[:], ident_h[:], ident_h[:], start=True, stop=True
            )


    # ---- background work: projection chains, emitted in small units ------
    # Each chain is split into units of 4 matmuls; the last unit also emits
    # the PSUM->SBUF cast. Units are drained between attention matmuls so
    # the PE has independent work while the scalar engine runs exp.
    bg = deque()          # of (emit_fn, chain_name, is_last_unit)
    done = set()

    def add_chain(name, units):
        for i, u in enumerate(units):
            bg.append((u, name, i == len(units) - 1))

    def drain(n):
        for _ in range(n):
            if not bg:
                return
            u, nm, last = bg.popleft()
            u()
            if last:
                done.add(nm)

    def require(name):
        while name not in done:
            assert bg, f"chain {name} needed but not queued"
            u, nm, last = bg.popleft()
            u()
            if last:
                done.add(nm)

    def proj_chain(m, t):
        """qkT[:, m*S + t-chunk] = (w col-block m)^T @ x^T, as 2 units."""
        st = {}

        def u1():
            st["ps"] = ps_small.tile([P, CH], F32, tag="small", name="pp_ps")
            for k in range(4):
                nc.tensor.matmul(
                    st["ps"][:],
                    w_all[:, k * COLS + m * P: k * COLS + (m + 1) * P],
                    xT[:, k * S + t * CH: k * S + (t + 1) * CH],
                    start=(k == 0),
                    stop=False,
                )

        def u2():
            for k in range(4, KC):
                nc.tensor.matmul(
                    st["ps"][:],
                    w_all[:, k * COLS + m * P: k * COLS + (m + 1) * P],
                    xT[:, k * S + t * CH: k * S + (t + 1) * CH],
                    start=False,
                    stop=(k == KC - 1),
                )
            nc.vector.tensor_copy(
                qkT[:, m * S + t * CH: m * S + (t + 1) * CH], st["ps"][:]
            )

        return [u1, u2]

    def v_chain(sb):
        """vaug[s-block sb] = x[sb] @ w_v (natural layout), as 2 units."""
        st = {}

        def u1():
            st["ps"] = ps_small.tile([P, CH], F32, tag="small", name="pv")
            for k in range(4):
                nc.tensor.matmul(
                    st["ps"][:, 0:256],
                    xT[:, k * S + sb * P: k * S + (sb + 1) * P],
                    w_all[:, k * COLS + 512: k * COLS + 768],
                    start=(k == 0),
                    stop=False,
                )

        def u2():
            for k in range(4, KC):
                nc.tensor.matmul(
                    st["ps"][:, 0:256],
                    xT[:, k * S + sb * P: k * S + (sb + 1) * P],
                    w_all[:, k * COLS + 512: k * COLS + 768],
                    start=False,
                    stop=(k == KC - 1),
                )
            nc.vector.tensor_copy(
                vaug[:, sb * HPC * 65:(sb + 1) * HPC * 65]
                .rearrange("p (g c) -> p g c", c=65)[:, :, 0:64],
                st["ps"][:, 0:256].rearrange("p (g c) -> p g c", c=64),
            )

        return [u1, u2]

    out_view = out_sb[:].rearrange("p (i g d) -> p i g d", g=HPC, d=HD)

    def finalize_head(h, t, po_h):
        """Transpose outT to natural layout, divide by the denominator."""
        osb_t = osbp.tile([65, CH], F16, tag="osb")
        nc.vector.tensor_copy(osb_t[:], po_h[:])
        fin32 = ps_small.tile([P, CH], F32, tag="small", name="fin32")
        fin = fin32.bitcast(F16)[:, 0:CH]
        for b in range(4):
            nc.tensor.transpose(
                fin[:, b * P:b * P + 65],
                osb_t[:, b * P:(b + 1) * P],
                ident_h[0:65, 0:65],
            )
        fin_view = fin[:, 0:CH].rearrange("p (n c) -> p n c", c=P)
        rc = rcp.tile([P, 4], F32, tag="rc")
        nc.vector.reciprocal(rc[:], fin_view[:, :, 64])
        nc.vector.tensor_mul(
            out_view[:, 4 * t:4 * t + 4, h, :],
            fin_view[:, :, 0:64],
            rc[:].broadcast_to([P, 4, HD]),
        )

    def attn_pair(pair, t, budget, carry):
        """Heads 2*pair, 2*pair+1; query chunk t (i in [512t, 512t+512)).

        `carry` is a list of closures (previous pair's finalize + output
        DMAs) run after this pair's first score/exp is emitted, so the
        scalar engine starts the next exp stream during the finalize."""
        hA, hB = 2 * pair, 2 * pair + 1
        qm, km = pair, 2 + pair
        require(f"q{pair}t{t}")
        po_a = ps_o.tile([65, CH], F32, tag="o")
        po_b = ps_o.tile([65, CH], F32, tag="o")
        po = {hA: po_a, hB: po_b}
        njb = 4 * t + 4
        pending = deque()  # (jb, p_t) AVs, emitted two slots behind

        def emit_av(jb, p_t, last):
            doff = jb - 4 * t
            off = max(0, P * doff)
            require(f"v{jb}")
            for hi, h in enumerate((hA, hB)):
                nc.tensor.matmul(
                    po[h][:, off:CH],
                    vaug[:, (jb * HPC + h) * 65: (jb * HPC + h + 1) * 65],
                    p_t[:, hi * CH + off:(hi + 1) * CH],
                    start=(jb == 0),
                    stop=last,
                )

        for jb in range(njb):
            if jb >= 4 * t:
                require(f"k{pair}t{t}")
            st = ps_st.tile([P, 2 * CH], F32, tag="st")
            for hi, h in enumerate((hA, hB)):
                hb = (h % 2) * 64
                nc.tensor.matmul(
                    st[:, hi * CH:(hi + 1) * CH],
                    qkT[hb:hb + 64, km * S + jb * P: km * S + (jb + 1) * P],
                    qkT[hb:hb + 64, qm * S + t * CH: qm * S + (t + 1) * CH],
                    start=True,
                    stop=True,
                    tile_position=(hb, 0),
                )
            p_t = pp.tile([P, 2 * CH], F16, tag="p")
            doff = jb - 4 * t
            off = max(0, P * doff)
            if off > 0:
                # skip exp of the fully-masked leading columns of both heads
                st_v = st[:].rearrange("p (h w) -> p h w", h=2)[:, :, off:CH]
                p_v = p_t[:].rearrange("p (h w) -> p h w", h=2)[:, :, off:CH]
            else:
                st_v, p_v = st[:], p_t[:]
            nc.scalar.activation(
                p_v, st_v, mybir.ActivationFunctionType.Exp,
                scale=float(SCALE),
            )
            if doff >= 0:
                # triangular mask on the diagonal 128x128 sub-block only;
                # the fully-masked zone is skipped by exp + AV column ranges.
                for hi in range(2):
                    c0 = hi * CH + off
                    sl = p_t[:, c0:c0 + P]
                    nc.gpsimd.affine_select(
                        out=sl,
                        in_=sl,
                        compare_op=mybir.AluOpType.is_ge,
                        fill=0.0,
                        base=0,
                        channel_multiplier=-1,
                        pattern=[[1, P]],
                    )
            if jb == 0:
                for f in carry:
                    f()
                carry = []
            drain(budget)
            pending.append((jb, p_t))
            if len(pending) > 2:
                ajb, ap_t = pending.popleft()
                emit_av(ajb, ap_t, last=False)
        while pending:
            ajb, ap_t = pending.popleft()
            emit_av(ajb, ap_t, last=(not pending))
        return [
            (lambda h=h: finalize_head(h, t, po[h])) for h in (hA, hB)
        ]

    # ---- prologue: chains needed to start attn(pair0, t0) ----------------
    add_chain("q0t0", proj_chain(0, 0))
    add_chain("k0t0", proj_chain(2, 0))
    add_chain("v0", v_chain(0))
    require("q0t0")
    require("k0t0")
    require("v0")

    carry = []
    for t in range(NT):
        # this phase's own late-deadline chains (diag v blocks, pair-1 q/k)
        for sb in range(4 * t + (1 if t == 0 else 0), 4 * t + 4):
            add_chain(f"v{sb}", v_chain(sb))
        add_chain(f"q1t{t}", proj_chain(1, t))
        add_chain(f"k1t{t}", proj_chain(3, t))
        # next phase's pair-0 q/k (needed at its very first slot)
        if t + 1 < NT:
            add_chain(f"q0t{t + 1}", proj_chain(0, t + 1))
            add_chain(f"k0t{t + 1}", proj_chain(2, t + 1))
        budget = 1
        carry = attn_pair(0, t, budget, carry)
        carry = attn_pair(1, t, budget, carry)

        def out_dma(t=t):
            for b in range(4):
                ib = 4 * t + b
                nc.sync.dma_start(
                    o_d[ib * P:(ib + 1) * P, :],
                    out_sb[:, ib * HPC * HD:(ib + 1) * HPC * HD],
                )

        carry.append(out_dma)
    for f in carry:
        f()
    assert not bg, f"{len(bg)} background units left unemitted"


def build_program():
    nc = bacc.Bacc(
        "TRN2",
        target_bir_lowering=False,
        debug=False,
        enable_asserts=False,
    )
    x_d = nc.dram_tensor("x", [D, S], F16, kind="ExternalInput").ap()
    w_d = nc.dram_tensor("w", [D, COLS], F16, kind="ExternalInput").ap()
    o_d = nc.dram_tensor("o", [S, HPC * HD], F32, kind="ExternalOutput").ap()

    with tile.TileContext(nc) as tc, ExitStack() as ctx:
        _build_body(ctx, tc, x_d, w_d, o_d)
    nc.compile()
    return nc


_CACHE = {}


def _compiled():
    if "nc" not in _CACHE:
        _CACHE["nc"] = build_program()
    return _CACHE["nc"]


def make_in_maps(x, w_qkv):
    x = np.asarray(x, dtype=np.float32)
    w_qkv = np.asarray(w_qkv, dtype=np.float32)
    # one transpose+cast per batch; cores sharing a batch reuse the array
    xT16 = [x[b].T.astype(np.float16) for b in range(B)]
    in_maps = []
    for c in range(NCORES):
        b = c // 4
        cs = (c % 4) * HPC * HD
        w_slice = np.concatenate(
            [
                w_qkv[:, cs:cs + HPC * HD],
                w_qkv[:, D + cs:D + cs + HPC * HD],
                w_qkv[:, 2 * D + cs:2 * D + cs + HPC * HD],
            ],
            axis=1,
        )
        in_maps.append(
            {
                "x": xT16[b],
                "w": np.ascontiguousarray(w_slice).astype(np.float16),
            }
        )
    return in_maps


def gather_out(results):
    out = np.empty((B, S, D), np.float32)
    for c in range(NCORES):
        b = c // 4
        cs = (c % 4) * HPC * HD
        out[b][:, cs:cs + HPC * HD] = results[c]["o"]
    return out


def kernel(x, w_qkv, w_o=None, **_):
    nc = _compiled()
    res = run_bass_kernel_spmd(nc, make_in_maps(x, w_qkv), core_ids=list(range(NCORES)))
    return gather_out(res.results)
